# revision 4
# baseline (speedup 1.0000x reference)
"""Trainium2 Bass kernel for the rank-weighted hard-negative hinge loss.

Math (reference):
    scores = im @ s.T                         # [N, N]
    diag   = diagonal(scores)
    rank1[i] = #{j : scores[i,j] < diag[i]}   (row rank of diag)
    rank2[j] = #{i : scores[i,j] < diag[j]}   (col rank of diag)
    cost_s  = 1/(rank1+1) * max_j!=i relu(M + scores[i,j] - diag[i])
    cost_im = 1/(rank2+1) * max_i!=j relu(M + scores[i,j] - diag[j])
    loss = sum(cost_s) + sum(cost_im)

Key identities used on-device:
    max_j relu(M + x_j - d) = relu(M + max_j x_j - d)   (relu/+const monotone)
so each core only needs, per row/column of its score block:
    - the masked row/col max of raw scores
    - the rank counts, recovered from sum_j sign(d - x_j)
The diagonal is excluded by adding -1e30 to the (i,i) entries of the PSUM
score block; the masked entry then deterministically counts as "below diag",
which exactly yields rank+1 (= the weight denominator).

Sharding: core r owns rows [r*1024, (r+1)*1024). To keep a single SPMD
program, each core receives s.T with columns rotated left by r*1024 so the
diagonal block sits at local column offset = local row index on every core.
Column stats are un-rotated on the host, which also does the final (tiny)
reduction across cores.
"""

import os
import numpy as np

N = 8192
D = 256
NCORES = 8
RL = N // NCORES  # rows per core
MARGIN = 0.2
NEG = np.float32(-1.0e30)

# matmul input dtype: "f32" (exact, 4 cyc/row) or "f32r" (full rate, to be
# validated numerically on hardware)
MM_DT = os.environ.get("BASS_MM_DT", "f32")

_cache = {}


def _build_nc():
    import concourse.bacc as bacc
    import concourse.mybir as mybir
    from concourse.tile import TileContext

    f32 = mybir.dt.float32
    mmdt = mybir.dt.float32r if MM_DT == "f32r" else mybir.dt.float32

    Sign = mybir.ActivationFunctionType.Sign
    AX = mybir.AxisListType.X
    MAX = mybir.AluOpType.max
    ADD = mybir.AluOpType.add

    nc = bacc.Bacc(None)

    imT = nc.declare_dram_parameter("imT", [D, RL], f32, isOutput=False)
    sT = nc.declare_dram_parameter("sT", [D, N], f32, isOutput=False)
    diag_r = nc.declare_dram_parameter("diag_r", [128, 8], f32, isOutput=False)
    diag_c = nc.declare_dram_parameter("diag_c", [128, 64], f32, isOutput=False)
    negeye = nc.declare_dram_parameter("negeye", [128, 128], f32, isOutput=False)
    s1_o = nc.declare_dram_parameter("s1", [128, 32], f32, isOutput=True)
    rmax_o = nc.declare_dram_parameter("rmax", [128, 32], f32, isOutput=True)
    s2_o = nc.declare_dram_parameter("s2", [128, 64], f32, isOutput=True)
    cmax_o = nc.declare_dram_parameter("cmax", [128, 64], f32, isOutput=True)

    with TileContext(nc) as tc:
        with (
            tc.tile_pool(name="consts", bufs=1) as cpool,
            tc.tile_pool(name="data", bufs=1) as dpool,
            tc.tile_pool(name="ps", bufs=2, space="PSUM") as pspool,
            tc.tile_pool(name="scratch", bufs=2) as tpool,
            tc.tile_pool(name="outs", bufs=1) as opool,
        ):
            t_negeye = cpool.tile([128, 128], f32, tag="negeye")
            nc.sync.dma_start(out=t_negeye[:], in_=negeye[:])
            t_dr = cpool.tile([128, 8], f32, tag="dr")
            nc.sync.dma_start(out=t_dr[:], in_=diag_r[:])
            t_dc = cpool.tile([128, 64], f32, tag="dc")
            nc.sync.dma_start(out=t_dc[:], in_=diag_c[:])

            t_imT = []
            for k in range(2):
                t = dpool.tile([128, RL], mmdt, tag=f"imT{k}")
                nc.sync.dma_start(out=t[:], in_=imT[k * 128:(k + 1) * 128, :])
                t_imT.append(t)
            t_sT = {}
            for b in range(4):
                for k in range(2):
                    t = dpool.tile([128, 2048], mmdt, tag=f"sT{k}_{b}")
                    nc.sync.dma_start(
                        out=t[:],
                        in_=sT[k * 128:(k + 1) * 128, b * 2048:(b + 1) * 2048],
                    )
                    t_sT[(k, b)] = t

            t_s1 = opool.tile([128, 32], f32, tag="s1")
            t_rmax = opool.tile([128, 32], f32, tag="rmax")
            t_s2 = opool.tile([128, 64], f32, tag="s2")
            t_cmax = opool.tile([128, 64], f32, tag="cmax")

            # ---- A phase: im rows on partitions; row stats ----
            for t in range(8):
                for sc in range(4):
                    ps = pspool.tile([128, 2048], f32, tag="ps")
                    for k in range(2):
                        for c in range(4):
                            nc.tensor.matmul(
                                ps[:, c * 512:(c + 1) * 512],
                                lhsT=t_imT[k][:, t * 128:(t + 1) * 128],
                                rhs=t_sT[(k, sc)][:, c * 512:(c + 1) * 512],
                                start=(k == 0),
                                stop=(k == 1),
                            )
                    if sc == 0:
                        off = t * 128
                        nc.vector.tensor_tensor(
                            ps[:, off:off + 128], ps[:, off:off + 128],
                            t_negeye[:], ADD,
                        )
                    idx = t * 4 + sc
                    trash = tpool.tile([128, 2048], f32, tag="trash")
                    nc.scalar.activation(
                        trash[:], ps[:], Sign,
                        bias=t_dr[:, t:t + 1], scale=-1.0,
                        accum_out=t_s1[:, idx:idx + 1],
                    )
                    nc.vector.tensor_reduce(
                        t_rmax[:, idx:idx + 1], ps[:], AX, MAX,
                    )

            # ---- B phase: s rows (columns of scores) on partitions ----
            for u in range(64):
                b, o = (u * 128) // 2048, (u * 128) % 2048
                ps = pspool.tile([128, 1024], f32, tag="ps")
                for k in range(2):
                    for c in range(2):
                        nc.tensor.matmul(
                            ps[:, c * 512:(c + 1) * 512],
                            lhsT=t_sT[(k, b)][:, o:o + 128],
                            rhs=t_imT[k][:, c * 512:(c + 1) * 512],
                            start=(k == 0),
                            stop=(k == 1),
                        )
                if u < 8:
                    off = u * 128
                    nc.vector.tensor_tensor(
                        ps[:, off:off + 128], ps[:, off:off + 128],
                        t_negeye[:], ADD,
                    )
                trash = tpool.tile([128, 1024], f32, tag="trash")
                nc.scalar.activation(
                    trash[:], ps[:], Sign,
                    bias=t_dc[:, u:u + 1], scale=-1.0,
                    accum_out=t_s2[:, u:u + 1],
                )
                nc.vector.tensor_reduce(
                    t_cmax[:, u:u + 1], ps[:], AX, MAX,
                )

            nc.sync.dma_start(out=s1_o[:], in_=t_s1[:])
            nc.sync.dma_start(out=rmax_o[:], in_=t_rmax[:])
            nc.sync.dma_start(out=s2_o[:], in_=t_s2[:])
            nc.sync.dma_start(out=cmax_o[:], in_=t_cmax[:])

    nc.finalize()
    return nc


def _get_nc():
    if "nc" not in _cache:
        _cache["nc"] = _build_nc()
    return _cache["nc"]


def make_in_maps(im, s):
    im = np.ascontiguousarray(np.asarray(im, dtype=np.float32))
    s = np.ascontiguousarray(np.asarray(s, dtype=np.float32))
    diag = np.einsum("ij,ij->i", im, s).astype(np.float32)
    sT_full = np.ascontiguousarray(s.T)
    negeye = np.where(np.eye(128, dtype=bool), NEG, np.float32(0.0)).astype(np.float32)
    in_maps = []
    for r in range(NCORES):
        lo = r * RL
        in_maps.append({
            "imT": np.ascontiguousarray(im[lo:lo + RL].T),
            "sT": np.ascontiguousarray(np.roll(sT_full, -lo, axis=1)),
            "diag_r": np.ascontiguousarray(diag[lo:lo + RL].reshape(8, 128).T),
            "diag_c": np.ascontiguousarray(np.roll(diag, -lo).reshape(64, 128).T),
            "negeye": negeye,
        })
    return in_maps, diag


def finish(results, diag):
    """Host-side reduction of the per-core stats to the scalar loss."""
    diag64 = diag.astype(np.float64)
    total = 0.0
    s2_sum = np.zeros(N, dtype=np.float64)
    cmax_g = np.full(N, -np.inf, dtype=np.float64)
    for r in range(NCORES):
        lo = r * RL
        s1 = results[r]["s1"].astype(np.float64)
        rmax = results[r]["rmax"].astype(np.float64)
        s2 = results[r]["s2"].astype(np.float64)
        cmax = results[r]["cmax"].astype(np.float64)
        # s1/rmax: [128(p), 32(t*4+sc)] ; local row i = t*128 + p
        s1sum = s1.reshape(128, 8, 4).sum(axis=2)
        rmax_row = rmax.reshape(128, 8, 4).max(axis=2)
        cnt1 = (N + s1sum.T.reshape(RL)) / 2.0  # = rank1 + 1 (mask counts once)
        rmaxv = rmax_row.T.reshape(RL)
        d_loc = diag64[lo:lo + RL]
        total += np.sum(np.maximum(MARGIN + rmaxv - d_loc, 0.0) / cnt1)
        # s2/cmax: [128(p), 64(u)] ; rotated col j' = u*128 + p -> global
        jj = (lo + np.arange(N)) % N
        s2_sum[jj] += s2.T.reshape(N)
        cmax_g[jj] = np.maximum(cmax_g[jj], cmax.T.reshape(N))
    cnt2 = (N + s2_sum) / 2.0  # = rank2 + 1
    total += np.sum(np.maximum(MARGIN + cmax_g - diag64, 0.0) / cnt2)
    return np.array(total, dtype=np.float32)


def run_on_hw(im, s, trace=False):
    from concourse.bass_utils import run_bass_kernel_spmd

    in_maps, diag = make_in_maps(im, s)
    nc = _get_nc()
    out = run_bass_kernel_spmd(nc, in_maps, list(range(NCORES)), trace=trace)
    return finish(out.results, diag), out


def kernel(im, s):
    result, _ = run_on_hw(im, s, trace=False)
    return result
